# revision 2
# baseline (speedup 1.0000x reference)
"""Invariant Point Attention on 8 Trainium2 NeuronCores.

Sequence-parallel sharding per spec hint: residues i (query dim) are split
across the 8 cores; params and k/v-side tensors are replicated; the pair
tensor z (the dominant memory term) is sharded along its first axis. Each
core computes its 96 rows of attention against the full keys/values, then
the row-shards are gathered into the full [768, 384] output.
"""

import math
import functools

import numpy as np
import jax
import jax.numpy as jnp
from jax.sharding import Mesh, NamedSharding, PartitionSpec as P
from jax.experimental.shard_map import shard_map

jax.config.update("jax_default_matmul_precision", "highest")

N = 768
C_S, C_Z, C_H, H, PQ, PV = 384, 128, 16, 12, 4, 8
INF, EPS = 100000.0, 1e-8
NCORES = 8
M = N // NCORES  # rows of i per core

_ORDER = [
    "s", "z", "rot", "trans", "mask", "Wq", "bq", "Wkv", "bkv", "Wqp", "bqp",
    "Wkvp", "bkvp", "Wb", "bb", "head_weights", "Wout", "bout",
]


def _points(s, W, b, rot, trans, n_rows, n_pts):
    p = s @ W.T + b
    p = jnp.stack(jnp.split(p, 3, axis=-1), axis=-1)  # [n_rows, H*n_pts, 3]
    p = jnp.einsum("nxy,npy->npx", rot, p) + trans[:, None, :]
    return p.reshape(n_rows, H, n_pts, 3)


def _ipa_shard(s, z, rot, trans, mask, Wq, bq, Wkv, bkv, Wqp, bqp, Wkvp, bkvp,
               Wb, bb, hw, Wout, bout):
    # z is the local shard [M, N, C_Z]; everything else is full/replicated.
    idx = jax.lax.axis_index("x")
    r0 = idx * M
    s_own = jax.lax.dynamic_slice_in_dim(s, r0, M, 0)
    rot_own = jax.lax.dynamic_slice_in_dim(rot, r0, M, 0)
    trans_own = jax.lax.dynamic_slice_in_dim(trans, r0, M, 0)
    mask_own = jax.lax.dynamic_slice_in_dim(mask, r0, M, 0)

    q = (s_own @ Wq.T + bq).reshape(M, H, C_H)
    kv = (s @ Wkv.T + bkv).reshape(N, H, 2 * C_H)
    k, v = kv[..., :C_H], kv[..., C_H:]

    q_pts = _points(s_own, Wqp, bqp, rot_own, trans_own, M, PQ)
    kv_pts = _points(s, Wkvp, bkvp, rot, trans, N, PQ + PV)
    k_pts, v_pts = kv_pts[..., :PQ, :], kv_pts[..., PQ:, :]

    b_bias = z @ Wb.T + bb  # [M, N, H]

    a = jnp.einsum("ihc,jhc->hij", q, k) * math.sqrt(1.0 / (3 * C_H))
    a = a + math.sqrt(1.0 / 3) * jnp.transpose(b_bias, (2, 0, 1))

    diff = q_pts[:, None] - k_pts[None]  # [M, N, H, PQ, 3]
    pt_att = jnp.sum(diff * diff, axis=-1)
    pt_att = jnp.sum(pt_att * hw[None, None, :, None], axis=-1) * (-0.5)

    square_mask = INF * (mask_own[:, None] * mask[None, :] - 1.0)
    a = a + jnp.transpose(pt_att, (2, 0, 1)) + square_mask[None]
    a = jax.nn.softmax(a, axis=-1)  # [H, M, N]

    o = jnp.einsum("hij,jhc->ihc", a, v).reshape(M, H * C_H)

    o_pt = jnp.einsum("hij,jhpx->ihpx", a, v_pts)  # [M, H, PV, 3]
    o_pt = jnp.einsum("nxy,nhpx->nhpy", rot_own, o_pt - trans_own[:, None, None, :])
    o_pt_norm = jnp.sqrt(jnp.sum(o_pt ** 2, axis=-1) + EPS).reshape(M, H * PV)
    o_pt = o_pt.reshape(M, H * PV, 3)

    o_pair = jnp.einsum("hij,ijc->ihc", a, z).reshape(M, H * C_Z)

    cat = jnp.concatenate(
        [o, o_pt[..., 0], o_pt[..., 1], o_pt[..., 2], o_pt_norm, o_pair], axis=-1)
    return cat @ Wout.T + bout


@functools.lru_cache(maxsize=1)
def _build():
    mesh = Mesh(np.array(jax.devices()[:NCORES]), ("x",))
    zspec = P("x", None, None)
    rep = P()
    in_specs = tuple(zspec if k == "z" else rep for k in _ORDER)
    out_spec = P("x", None)
    fn = shard_map(_ipa_shard, mesh=mesh, in_specs=in_specs,
                   out_specs=out_spec, check_rep=False)
    in_sh = tuple(NamedSharding(mesh, sp) for sp in in_specs)
    jitted = jax.jit(fn, in_shardings=in_sh,
                     out_shardings=NamedSharding(mesh, out_spec))
    return jitted


def kernel(**inputs) -> np.ndarray:
    inputs = dict(inputs)
    hwv = np.asarray(inputs["head_weights"], dtype=np.float64)
    # host-side weight preprocessing: softplus(head_weights) * point-att scale
    inputs["head_weights"] = (np.log1p(np.exp(hwv))
                              * math.sqrt(1.0 / (3 * (PQ * 9.0 / 2)))).astype(np.float32)
    args = [np.asarray(inputs[k]) for k in _ORDER]
    jitted = _build()
    out = jitted(*args)
    return np.asarray(out, dtype=np.float32)


if __name__ == "__main__":
    rng = np.random.default_rng(0)
    fake = {}
    fake["s"] = rng.standard_normal((N, C_S), dtype=np.float32)
    fake["z"] = rng.standard_normal((N, N, C_Z), dtype=np.float32)
    fake["rot"] = rng.standard_normal((N, 3, 3), dtype=np.float32)
    fake["trans"] = rng.standard_normal((N, 3), dtype=np.float32)
    fake["mask"] = np.ones((N,), np.float32)
    sc = 0.02
    fake["Wq"] = rng.standard_normal((H * C_H, C_S), dtype=np.float32) * sc
    fake["bq"] = np.zeros((H * C_H,), np.float32)
    fake["Wkv"] = rng.standard_normal((2 * H * C_H, C_S), dtype=np.float32) * sc
    fake["bkv"] = np.zeros((2 * H * C_H,), np.float32)
    fake["Wqp"] = rng.standard_normal((H * PQ * 3, C_S), dtype=np.float32) * sc
    fake["bqp"] = np.zeros((H * PQ * 3,), np.float32)
    fake["Wkvp"] = rng.standard_normal((H * (PQ + PV) * 3, C_S), dtype=np.float32) * sc
    fake["bkvp"] = np.zeros((H * (PQ + PV) * 3,), np.float32)
    fake["Wb"] = rng.standard_normal((H, C_Z), dtype=np.float32) * sc
    fake["bb"] = np.zeros((H,), np.float32)
    fake["head_weights"] = rng.standard_normal((H,), dtype=np.float32) * 0.1
    cat_dim = H * (C_Z + C_H + PV * 4)
    fake["Wout"] = rng.standard_normal((C_S, cat_dim), dtype=np.float32) * sc
    fake["bout"] = np.zeros((C_S,), np.float32)
    out = kernel(**fake)
    print("smoke ok", out.shape, out.dtype, float(np.abs(out).max()))


# revision 3
# speedup vs baseline: 72.8316x; 72.8316x over previous
"""Invariant Point Attention on 8 Trainium2 NeuronCores.

Sequence-parallel sharding per spec hint: residues i (query dim) are split
across the 8 cores; params and k/v-side tensors are replicated; the pair
tensor z (the dominant memory term) is sharded along its first axis. Each
core computes its 96 rows of attention against the full keys/values, then
the row-shards are gathered into the full [768, 384] output.
"""

import math
import functools

import numpy as np
import jax
import jax.numpy as jnp
from jax.sharding import Mesh, NamedSharding, PartitionSpec as P
from jax.experimental.shard_map import shard_map

jax.config.update("jax_default_matmul_precision", "highest")

N = 768
C_S, C_Z, C_H, H, PQ, PV = 384, 128, 16, 12, 4, 8
INF, EPS = 100000.0, 1e-8
NCORES = 8
M = N // NCORES  # rows of i per core

_ORDER = [
    "s", "z", "rot", "trans", "mask", "Wq", "bq", "Wkv", "bkv", "Wqp", "bqp",
    "Wkvp", "bkvp", "Wb", "bb", "head_weights", "Wout", "bout",
]


def _points(s, W, b, rot, trans, n_rows, n_pts):
    p = s @ W.T + b
    p = jnp.stack(jnp.split(p, 3, axis=-1), axis=-1)  # [n_rows, H*n_pts, 3]
    p = jnp.einsum("nxy,npy->npx", rot, p) + trans[:, None, :]
    return p.reshape(n_rows, H, n_pts, 3)


def _ipa_shard(s, z, rot, trans, mask, Wq, bq, Wkv, bkv, Wqp, bqp, Wkvp, bkvp,
               Wb, bb, hw, Wout, bout):
    # z is the local shard [M, N, C_Z]; everything else is full/replicated.
    idx = jax.lax.axis_index("x")
    r0 = idx * M
    s_own = jax.lax.dynamic_slice_in_dim(s, r0, M, 0)
    rot_own = jax.lax.dynamic_slice_in_dim(rot, r0, M, 0)
    trans_own = jax.lax.dynamic_slice_in_dim(trans, r0, M, 0)
    mask_own = jax.lax.dynamic_slice_in_dim(mask, r0, M, 0)

    q = (s_own @ Wq.T + bq).reshape(M, H, C_H)
    kv = (s @ Wkv.T + bkv).reshape(N, H, 2 * C_H)
    k, v = kv[..., :C_H], kv[..., C_H:]

    q_pts = _points(s_own, Wqp, bqp, rot_own, trans_own, M, PQ)
    kv_pts = _points(s, Wkvp, bkvp, rot, trans, N, PQ + PV)
    k_pts, v_pts = kv_pts[..., :PQ, :], kv_pts[..., PQ:, :]

    b_bias = z @ Wb.T + bb  # [M, N, H]

    a = jnp.einsum("ihc,jhc->hij", q, k) * math.sqrt(1.0 / (3 * C_H))
    a = a + math.sqrt(1.0 / 3) * jnp.transpose(b_bias, (2, 0, 1))

    # point-distance term via |q|^2 + |k|^2 - 2 q.k (matmul form, no 5-D tensor)
    cross = jnp.einsum("ihpx,jhpx->hij", q_pts, k_pts)
    qn = jnp.sum(q_pts ** 2, axis=(-1, -2))  # [M, H]
    kn = jnp.sum(k_pts ** 2, axis=(-1, -2))  # [N, H]
    pt_att = qn.T[:, :, None] + kn.T[:, None, :] - 2.0 * cross  # [H, M, N]
    pt_att = pt_att * (-0.5) * hw[:, None, None]

    square_mask = INF * (mask_own[:, None] * mask[None, :] - 1.0)
    a = a + pt_att + square_mask[None]
    a = jax.nn.softmax(a, axis=-1)  # [H, M, N]

    o = jnp.einsum("hij,jhc->ihc", a, v).reshape(M, H * C_H)

    o_pt = jnp.einsum("hij,jhpx->ihpx", a, v_pts)  # [M, H, PV, 3]
    o_pt = jnp.einsum("nxy,nhpx->nhpy", rot_own, o_pt - trans_own[:, None, None, :])
    o_pt_norm = jnp.sqrt(jnp.sum(o_pt ** 2, axis=-1) + EPS).reshape(M, H * PV)
    o_pt = o_pt.reshape(M, H * PV, 3)

    o_pair = jnp.einsum("hij,ijc->ihc", a, z).reshape(M, H * C_Z)

    cat = jnp.concatenate(
        [o, o_pt[..., 0], o_pt[..., 1], o_pt[..., 2], o_pt_norm, o_pair], axis=-1)
    return cat @ Wout.T + bout


@functools.lru_cache(maxsize=1)
def _build():
    mesh = Mesh(np.array(jax.devices()[:NCORES]), ("x",))
    zspec = P("x", None, None)
    rep = P()
    in_specs = tuple(zspec if k == "z" else rep for k in _ORDER)
    out_spec = P("x", None)
    fn = shard_map(_ipa_shard, mesh=mesh, in_specs=in_specs,
                   out_specs=out_spec, check_rep=False)
    in_sh = tuple(NamedSharding(mesh, sp) for sp in in_specs)
    jitted = jax.jit(fn, in_shardings=in_sh,
                     out_shardings=NamedSharding(mesh, out_spec))
    return jitted


def kernel(**inputs) -> np.ndarray:
    inputs = dict(inputs)
    hwv = np.asarray(inputs["head_weights"], dtype=np.float64)
    # host-side weight preprocessing: softplus(head_weights) * point-att scale
    inputs["head_weights"] = (np.log1p(np.exp(hwv))
                              * math.sqrt(1.0 / (3 * (PQ * 9.0 / 2)))).astype(np.float32)
    args = [np.asarray(inputs[k]) for k in _ORDER]
    jitted = _build()
    out = jitted(*args)
    return np.asarray(out, dtype=np.float32)


if __name__ == "__main__":
    rng = np.random.default_rng(0)
    fake = {}
    fake["s"] = rng.standard_normal((N, C_S), dtype=np.float32)
    fake["z"] = rng.standard_normal((N, N, C_Z), dtype=np.float32)
    fake["rot"] = rng.standard_normal((N, 3, 3), dtype=np.float32)
    fake["trans"] = rng.standard_normal((N, 3), dtype=np.float32)
    fake["mask"] = np.ones((N,), np.float32)
    sc = 0.02
    fake["Wq"] = rng.standard_normal((H * C_H, C_S), dtype=np.float32) * sc
    fake["bq"] = np.zeros((H * C_H,), np.float32)
    fake["Wkv"] = rng.standard_normal((2 * H * C_H, C_S), dtype=np.float32) * sc
    fake["bkv"] = np.zeros((2 * H * C_H,), np.float32)
    fake["Wqp"] = rng.standard_normal((H * PQ * 3, C_S), dtype=np.float32) * sc
    fake["bqp"] = np.zeros((H * PQ * 3,), np.float32)
    fake["Wkvp"] = rng.standard_normal((H * (PQ + PV) * 3, C_S), dtype=np.float32) * sc
    fake["bkvp"] = np.zeros((H * (PQ + PV) * 3,), np.float32)
    fake["Wb"] = rng.standard_normal((H, C_Z), dtype=np.float32) * sc
    fake["bb"] = np.zeros((H,), np.float32)
    fake["head_weights"] = rng.standard_normal((H,), dtype=np.float32) * 0.1
    cat_dim = H * (C_Z + C_H + PV * 4)
    fake["Wout"] = rng.standard_normal((C_S, cat_dim), dtype=np.float32) * sc
    fake["bout"] = np.zeros((C_S,), np.float32)
    out = kernel(**fake)
    print("smoke ok", out.shape, out.dtype, float(np.abs(out).max()))
